# revision 10
# baseline (speedup 1.0000x reference)
"""Paged causal GQA attention on 8 TRN2 NeuronCores.

Problem (hardcoded): B=8 seqs x S=1024 tokens, H=32 q-heads, KVH=8 kv-heads
(GQA group 4), D=128, f32 in/out, paged KV cache (block_size 16, 512 blocks).

Strategy:
  - Host side: scatter k/v into the paged cache via slot_mapping and gather
    per-sequence K/V via block_tables (pure permutation / shard preparation,
    exactly the reference semantics), then shard one sequence per core.
  - Device side (per core, SPMD): causal GQA attention for one sequence.
    Layout trick: compute scores^T [k, q] with K^T stationary so softmax'd
    probs P^T are directly the PV stationary operand (no P transpose), and
    append a ones-column to V so the softmax denominator falls out of the
    PV matmul. exp(scale*x) without max-subtraction (scores bounded ~|4.5|).
    bf16 matmul inputs, f32 PSUM accumulation.
  - Instruction-count hygiene: K/V staged once with 4KB DMA descriptors,
    Q staged per kv-group (2KB descriptors), exp batched across 2 PSUM
    banks, normalize via one broadcast-AP multiply per chunk, output DMA
    batched per (group, chunk).
"""

import numpy as np

B, S, H, KVH, D = 8, 1024, 32, 8, 128
G = H // KVH
NB, BS = 512, 16
T = B * S
SCALE = 0.08838834764831845
NCORES = 8

_compiled = {}


def _build():
    import concourse.bass as bass
    import concourse.bacc as bacc
    import concourse.mybir as mybir
    import concourse.tile as tile
    from concourse.masks import make_identity

    f32 = mybir.dt.float32
    bf16 = mybir.dt.bfloat16
    EXP = mybir.ActivationFunctionType.Exp

    nc = bacc.Bacc("TRN2", target_bir_lowering=False, debug=False,
                   num_devices=NCORES)
    qd = nc.dram_tensor("q", [S, H * D], f32, kind="ExternalInput").ap()
    kd = nc.dram_tensor("k", [S, KVH * D], f32, kind="ExternalInput").ap()
    vd = nc.dram_tensor("v", [S, KVH * D], f32, kind="ExternalInput").ap()
    od = nc.dram_tensor("out", [S, H * D], f32, kind="ExternalOutput").ap()

    NT = S // 128            # 8 k/q tiles of 128
    CB = 4                   # q-blocks per chunk (chunk = 512 q cols)
    NCH = NT // CB           # chunks per head

    with tile.TileContext(nc) as tc:
        with (
            tc.tile_pool(name="const", bufs=1) as constp,
            tc.tile_pool(name="stage_kv", bufs=2) as stagekvp,
            tc.tile_pool(name="stage_q", bufs=2) as stageqp,
            tc.tile_pool(name="kb16", bufs=2) as kbp,
            tc.tile_pool(name="qb16", bufs=2) as qbp,
            tc.tile_pool(name="kt", bufs=2) as ktp,
            tc.tile_pool(name="va", bufs=2) as vap,
            tc.tile_pool(name="qt", bufs=9) as qtp,
            tc.tile_pool(name="pt", bufs=8) as ptp,
            tc.tile_pool(name="ost", bufs=2) as ostp,
            tc.tile_pool(name="small", bufs=4) as smallp,
            tc.tile_pool(name="psum_s", bufs=3, space="PSUM") as psum_s,
            tc.tile_pool(name="psum_o", bufs=1, space="PSUM") as psum_o,
        ):
            ident = constp.tile([128, 128], bf16, tag="ident")
            make_identity(nc, ident[:])

            # ---- stage K once (4KB descriptors); V after first Q ----
            Knat = stagekvp.tile([128, NT, KVH * D], f32, tag="stage_kv")
            nc.sync.dma_start(Knat[:], kd.rearrange("(n p) c -> p n c", p=128))
            Qnat0 = stageqp.tile([128, NT, G * D], f32, tag="stage_q")
            nc.sync.dma_start(
                Qnat0[:], qd[:, 0:512].rearrange("(n p) c -> p n c", p=128))
            Vnat = stagekvp.tile([128, NT, KVH * D], f32, tag="stage_kv")
            nc.sync.dma_start(Vnat[:], vd.rearrange("(n p) c -> p n c", p=128))

            def transpose_8(dst_1024, src_fn):
                # 8 PE transposes of [128,128] bf16 blocks -> dst [128, 1024]
                for half in range(2):
                    trp = psum_s.tile([128, 512], bf16, tag="st")
                    for jj in range(4):
                        nc.tensor.transpose(
                            trp[:, jj * 128:(jj + 1) * 128],
                            src_fn(half * 4 + jj), ident[:])
                    nc.vector.tensor_copy(
                        dst_1024[:, half * 512:(half + 1) * 512], trp[:])

            def prep(g):
                # per-group K^T / V-augmented / Q^T preparation
                kb = kbp.tile([128, NT, 128], bf16, tag="kb16")
                nc.vector.tensor_copy(kb[:], Knat[:, :, g * 128:(g + 1) * 128])
                KT = ktp.tile([128, S], bf16, tag="kt")
                transpose_8(KT, lambda j: kb[:, j, :])
                VA = vap.tile([128, NT, D + 1], bf16, tag="va")
                nc.gpsimd.memset(VA[:, :, D:D + 1], 1.0)
                nc.vector.tensor_copy(
                    VA[:, :, 0:D], Vnat[:, :, g * 128:(g + 1) * 128])
                if g == 0:
                    Qnat = Qnat0
                else:
                    Qnat = stageqp.tile([128, NT, G * D], f32, tag="stage_q")
                    nc.sync.dma_start(
                        Qnat[:],
                        qd[:, g * 512:(g + 1) * 512]
                        .rearrange("(n p) c -> p n c", p=128))
                Qb = qbp.tile([128, NT, G * D], bf16, tag="qb16")
                nc.vector.tensor_copy(Qb[:], Qnat[:])
                QTs = []
                for h4 in range(G):
                    QT = qtp.tile([128, S], bf16, tag="qt")
                    transpose_8(QT, lambda i: Qb[:, i, h4 * 128:(h4 + 1) * 128])
                    QTs.append(QT)
                return KT, VA, QTs

            # ---- main loop, software-pipelined prep ----
            cur = prep(0)
            for g in range(KVH):
                KT, VA, QTs = cur
                if g + 1 < KVH:
                    cur = prep(g + 1)

                for c in range(NCH):
                    i0 = c * CB
                    ost = ostp.tile([128, CB, G * D], f32, tag="ost")
                    for h4 in range(G):
                        QT = QTs[h4]
                        # o blocks at col offsets ii*256, width D+1; per-bank
                        # accumulation groups must not interleave, so each
                        # block's start..stop runs to completion.
                        o = psum_o.tile([128, 1024], f32, tag="o")
                        pts = {}
                        for p0 in range(0, i0 + CB, 2):
                            st = psum_s.tile([128, 1024], f32, tag="st")
                            off = 0
                            metas = []
                            for j in (p0, p0 + 1):
                                jj = j - i0
                                if jj < 0:
                                    n = CB * 128
                                    qcol = i0 * 128
                                else:
                                    n = (CB - jj) * 128
                                    qcol = j * 128
                                nc.tensor.matmul(
                                    st[:, off:off + n],
                                    lhsT=KT[:, j * 128:(j + 1) * 128],
                                    rhs=QT[:, qcol:qcol + n],
                                    start=True, stop=True,
                                )
                                metas.append((j, jj, off))
                                off += n
                            pt = ptp.tile([128, 1024], bf16, tag="pt")
                            nc.scalar.activation(pt[:, :off], st[:, :off],
                                                 EXP, scale=SCALE)
                            for (j, jj, o_) in metas:
                                if jj >= 0:
                                    # zero strictly-lower (q < k) of diag block
                                    nc.gpsimd.affine_select(
                                        out=pt[:, o_:o_ + 128],
                                        in_=pt[:, o_:o_ + 128],
                                        compare_op=mybir.AluOpType.is_ge,
                                        fill=0.0, base=0,
                                        pattern=[[1, 128]],
                                        channel_multiplier=-1,
                                    )
                                pts[j] = (pt, o_)
                        for ii in range(CB):
                            i = i0 + ii
                            for j in range(i + 1):
                                jj = j - i0
                                pt, o_ = pts[j]
                                col = o_ + (ii - max(jj, 0)) * 128
                                nc.tensor.matmul(
                                    o[:, ii * 256: ii * 256 + D + 1],
                                    lhsT=pt[:, col:col + 128],
                                    rhs=VA[:, j, :],
                                    start=(j == 0), stop=(j == i),
                                )
                        rec = smallp.tile([128, CB], f32, tag="rec")
                        nc.vector.reciprocal(rec[:], o[:, D::256])
                        ov = o[:].rearrange("p (b x) -> p b x", x=256)[:, :, 0:D]
                        rbc = (rec[:].rearrange("p b -> p b ()")
                               .broadcast_to((128, CB, D)))
                        nc.vector.tensor_tensor(
                            ost[:, :, h4 * 128:(h4 + 1) * 128], ov, rbc,
                            mybir.AluOpType.mult)
                    nc.sync.dma_start(
                        od[c * 512:(c + 1) * 512, g * 512:(g + 1) * 512]
                        .rearrange("(b p) d -> p b d", p=128),
                        ost[:],
                    )

    nc.compile()
    return nc


def _get_nc():
    if "nc" not in _compiled:
        _compiled["nc"] = _build()
    return _compiled["nc"]


def kernel(q, k, v, k_cache, v_cache, slot_mapping, block_tables):
    from concourse.bass_utils import run_bass_kernel_spmd

    q = np.ascontiguousarray(np.asarray(q, dtype=np.float32))
    k = np.asarray(k, dtype=np.float32)
    v = np.asarray(v, dtype=np.float32)
    sm = np.asarray(slot_mapping).astype(np.int64)
    bt = np.asarray(block_tables).astype(np.int64)

    # store_kvcache + page gather (reference semantics, pure permutation)
    kc = np.asarray(k_cache, dtype=np.float32).reshape(NB * BS, KVH * D).copy()
    vc = np.asarray(v_cache, dtype=np.float32).reshape(NB * BS, KVH * D).copy()
    kc[sm] = k
    vc[sm] = v
    kg = kc.reshape(NB, BS, KVH * D)[bt].reshape(B, S, KVH * D)
    vg = vc.reshape(NB, BS, KVH * D)[bt].reshape(B, S, KVH * D)
    qs = q.reshape(B, S, H * D)

    in_maps = [
        {"q": np.ascontiguousarray(qs[i]),
         "k": np.ascontiguousarray(kg[i]),
         "v": np.ascontiguousarray(vg[i])}
        for i in range(NCORES)
    ]
    nc = _get_nc()
    res = run_bass_kernel_spmd(nc, in_maps, core_ids=list(range(NCORES)))
    _compiled["last_result"] = res
    out = np.concatenate([res.results[i]["out"] for i in range(NCORES)], axis=0)
    return out.astype(np.float32)
